# revision 1
# baseline (speedup 1.0000x reference)
"""Single-head causal attention on 8 TRN2 NeuronCores, batch-parallel.

Problem: x[8,2048,1024] f32, Wq/Wk/Wv[1024,64] f32
  q,k,v = x@W*  ;  scores = q k^T / sqrt(1024), causal  ;  out = softmax(scores) @ v

Sharding: batch dim across 8 cores (1 batch element per core, no collectives).

Per-core dataflow (all matmuls fp32r = tf32-class, 1 cyc/row at N>=256):
  A) xT: PE-transpose x [2048,1024] -> xT [c=128part x 8ct, t=2048]
  B) proj: lhsT=[Wq|Wv] -> qvT psum [128,512] (rows 0:64 qT, 64:128 vT);
     kT separate (M=64).  vT -> PE-transpose -> v_aug [128s,16,65] with ones col.
  C) per t-chunk (512): st_j [s=128, t=512] = kT_j^T q ; exp(st/32) -> wst (f32r);
     causal mask on diagonal tiles (memset + 0/1 triangle multiply);
     psum_o [65,512] += v_aug_j^T wst_j  (row 64 = softmax denominator);
     PE-transpose psum_o -> [128t, 65], out = cols0:64 * recip(col 64) -> HBM.
"""

import numpy as np

import concourse.bacc as bacc
import concourse.mybir as mybir
import concourse.tile as tile
from concourse.bass_utils import run_bass_kernel_spmd

F32 = mybir.dt.float32
F32R = mybir.dt.float32r

B, T, C, H = 8, 2048, 1024, 64
NCT = C // 128          # 8 c-tiles
NTT = T // 128          # 16 t/s-tiles
NCH = T // 512          # 4 t-chunks
SCALE = float(C ** -0.5)

_CACHE = {}


def build():
    nc = bacc.Bacc(name="head_attn")
    x_d = nc.dram_tensor("x", [T, C], F32, kind="ExternalInput")
    wq_d = nc.dram_tensor("Wq", [C, H], F32, kind="ExternalInput")
    wk_d = nc.dram_tensor("Wk", [C, H], F32, kind="ExternalInput")
    wv_d = nc.dram_tensor("Wv", [C, H], F32, kind="ExternalInput")
    id_d = nc.dram_tensor("ident", [128, 128], F32, kind="ExternalInput")
    ih_d = nc.dram_tensor("identhi", [128, 64], F32, kind="ExternalInput")
    tri_d = nc.dram_tensor("tri", [128, 128], F32, kind="ExternalInput")
    msk_d = nc.dram_tensor("masks", [128, 4, 512], F32, kind="ExternalInput")
    one_d = nc.dram_tensor("ones16", [128, 16, 2], F32, kind="ExternalInput")
    out_d = nc.dram_tensor("out", [T, H], F32, kind="ExternalOutput")

    with tile.TileContext(nc) as tc:
        with (
            tc.tile_pool(name="singles", bufs=1) as singles,
            tc.tile_pool(name="stage", bufs=4) as stage,
            tc.tile_pool(name="work", bufs=8) as work,
            tc.tile_pool(name="outp", bufs=4) as outp,
            tc.tile_pool(name="pbig", bufs=4, space="PSUM") as pbig,
            tc.tile_pool(name="pacc", bufs=2, space="PSUM") as pacc,
            tc.tile_pool(name="psmall", bufs=2, space="PSUM") as psmall,
        ):
            # ---- constants / weights
            ident = singles.tile([128, 128], F32R)
            identhi = singles.tile([128, 64], F32R)
            tri = singles.tile([128, 128], F32R)
            nc.sync.dma_start(ident, id_d[:, :].bitcast(F32R))
            nc.sync.dma_start(identhi, ih_d[:, :].bitcast(F32R))
            nc.sync.dma_start(tri, tri_d[:, :].bitcast(F32R))
            masks = singles.tile([128, 4, 512], F32R)
            nc.sync.dma_start(masks, msk_d[:, :, :].bitcast(F32R))

            wqv = singles.tile([128, NCT, 128], F32R)   # [c | ct | (q h, v h)]
            wk = singles.tile([128, NCT, H], F32R)
            nc.sync.dma_start(
                wqv[:, :, 0:H], wq_d.rearrange("(t c) h -> c t h", c=128).bitcast(F32R))
            nc.sync.dma_start(
                wqv[:, :, H:128], wv_d.rearrange("(t c) h -> c t h", c=128).bitcast(F32R))
            nc.sync.dma_start(
                wk[:, :, :], wk_d.rearrange("(t c) h -> c t h", c=128).bitcast(F32R))

            xT = singles.tile([128, NCT, T], F32R)      # [c, ct, t]

            # ---- A) transpose x into xT
            for tt in range(NTT):
                xs = stage.tile([128, C], F32R, tag="xs")
                nc.sync.dma_start(xs, x_d[tt * 128:(tt + 1) * 128, :].bitcast(F32R))
                for cg in range(2):                     # 2 groups of 4 c-tiles
                    pt = pbig.tile([128, 512], F32R, tag="big")
                    for k in range(4):
                        ct = cg * 4 + k
                        nc.tensor.transpose(
                            pt[:, k * 128:(k + 1) * 128],
                            xs[:, ct * 128:(ct + 1) * 128], ident)
                    dst = xT[:, cg * 4:(cg + 1) * 4, tt * 128:(tt + 1) * 128]
                    if (2 * tt + cg) % 8 < 5:
                        nc.scalar.copy(dst, pt.rearrange("p (a b) -> p a b", a=4))
                    else:
                        nc.vector.tensor_copy(dst, pt.rearrange("p (a b) -> p a b", a=4))

            # ---- B) projections
            qvT = singles.tile([128, T], F32R)          # rows 0:64 qT, 64:128 vT
            kT = singles.tile([64, T], F32R)
            for i in range(NCH):
                pq = pbig.tile([128, 512], F32, tag="big")
                for ct in range(NCT):
                    nc.tensor.matmul(pq, wqv[:, ct, :], xT[:, ct, i * 512:(i + 1) * 512],
                                     start=(ct == 0), stop=(ct == NCT - 1))
                nc.vector.tensor_copy(qvT[:, i * 512:(i + 1) * 512].bitcast(F32R), pq)
                pk = pbig.tile([64, 512], F32, tag="big")
                for ct in range(NCT):
                    nc.tensor.matmul(pk, wk[:, ct, :], xT[:, ct, i * 512:(i + 1) * 512],
                                     start=(ct == 0), stop=(ct == NCT - 1))
                nc.vector.tensor_copy(kT[:, i * 512:(i + 1) * 512].bitcast(F32R), pk)

            # v_aug [s=128, 16, 66] with two ones columns (66 keeps the
            # fp32r output transpose even-sized and partition-base aligned)
            v_aug = singles.tile([128, NTT, 66], F32R)
            nc.sync.dma_start(v_aug[:, :, 64:66], one_d[:, :, :].bitcast(F32R))
            for s in range(NTT):
                pv = psmall.tile([128, 64], F32R, tag="small")
                nc.tensor.transpose(
                    pv, qvT[64:128, s * 128:(s + 1) * 128], identhi[64:128, :])
                nc.vector.tensor_copy(v_aug[:, s, 0:64], pv)

            # ---- C) attention
            for i in range(NCH):
                po = pacc.tile([66, 512], F32)
                nj = 4 * i + 4
                for j in range(nj):
                    pst = pbig.tile([128, 512], F32, tag="big")
                    nc.tensor.matmul(pst, kT[:, j * 128:(j + 1) * 128],
                                     qvT[0:64, i * 512:(i + 1) * 512],
                                     start=True, stop=True)
                    wst = work.tile([128, 512], F32R, tag="wst")
                    k = j - 4 * i
                    d = 128 * k if k > 0 else 0
                    nc.scalar.activation(wst[:, d:], pst[:, d:],
                                         mybir.ActivationFunctionType.Exp, scale=SCALE)
                    if k >= 0:                           # diagonal: mask the triangle
                        nc.vector.tensor_mul(wst[:, d:d + 128], wst[:, d:d + 128], tri)
                    nc.tensor.matmul(po[:, d:], v_aug[:, j, :], wst[:, d:],
                                     start=(j == 0), stop=(j == nj - 1))

                oT = outp.tile([66, 512], F32R, tag="oT")
                nc.scalar.copy(oT, po)
                for b in range(4):
                    pn = psmall.tile([128, 66], F32R, tag="small")
                    nc.tensor.transpose(pn, oT[:, b * 128:(b + 1) * 128],
                                        ident[0:66, 0:66])
                    rec = outp.tile([128, 1], F32, tag="rec")
                    nc.vector.reciprocal(rec, pn[:, 64:65])
                    ob = outp.tile([128, 64], F32, tag="ob")
                    nc.vector.tensor_scalar_mul(ob, pn[:, 0:64], rec)
                    nc.sync.dma_start(
                        out_d[i * 512 + b * 128: i * 512 + (b + 1) * 128, :], ob)

    nc.compile()
    return nc


def _consts():
    ident = np.eye(128, dtype=np.float32)
    identhi = np.zeros((128, 64), dtype=np.float32)
    identhi[64:128, :] = np.eye(64, dtype=np.float32)
    # tri[p, v] = 1 where v >= p  (valid, upper incl diag in [s, u] coords)
    tri = np.triu(np.ones((128, 128), dtype=np.float32))
    return ident, identhi, tri


def kernel(x, Wq, Wk, Wv, trace=False):
    x = np.ascontiguousarray(np.asarray(x, dtype=np.float32))
    Wq = np.ascontiguousarray(np.asarray(Wq, dtype=np.float32))
    Wk = np.ascontiguousarray(np.asarray(Wk, dtype=np.float32))
    Wv = np.ascontiguousarray(np.asarray(Wv, dtype=np.float32))

    if "nc" not in _CACHE:
        _CACHE["nc"] = build()
    nc = _CACHE["nc"]

    ident, identhi, tri = _consts()
    p = np.arange(128, dtype=np.float32)[:, None]
    u = np.arange(512, dtype=np.float32)[None, :]
    masks = np.stack([(u >= p + 128 * k).astype(np.float32) for k in range(4)], axis=1)
    ones16 = np.ones((128, 16, 2), dtype=np.float32)
    in_maps = [
        {"x": x[b], "Wq": Wq, "Wk": Wk, "Wv": Wv,
         "ident": ident, "identhi": identhi, "tri": tri,
         "masks": masks, "ones16": ones16}
        for b in range(B)
    ]
    try:
        res = run_bass_kernel_spmd(nc, in_maps, core_ids=list(range(B)), trace=trace)
    except ModuleNotFoundError:
        res = run_bass_kernel_spmd(nc, in_maps, core_ids=list(range(B)))
    out = np.stack([r["out"] for r in res.results], axis=0)
    kernel.last_exec_time_ns = res.exec_time_ns
    kernel.last_results = res
    return out



# revision 2
# speedup vs baseline: 1.0025x; 1.0025x over previous
"""Single-head causal attention on 8 TRN2 NeuronCores, batch-parallel (v3).

Problem: x[8,2048,1024] f32, Wq/Wk/Wv[1024,64] f32
  q,k,v = x@W*  ;  scores = q k^T / sqrt(1024), causal  ;  out = softmax(scores) @ v

Sharding: batch dim across 8 cores (1 batch element per core, no collectives).
Host prep: cast to bf16; weights packed [Wq|Wk|Wv] -> [128, 8, 192].

Per-core dataflow:
  A) x bf16 loaded TRANSPOSED via XBAR DMA transpose (t-quarters then halves,
     split across SP/Act issue queues) -> xT [c=128 x 8ct, t=2048].
  B) per t-chunk (512): packed qk proj (M=128) -> psum; DVE copies rows 0:64
     -> qS, rows 64:128 -> kS (partition-shifted), fp8e4 (zero 2nd DoubleRow
     k-tile) or bf16. v proj DIRECT in [s,h]: lhsT = xT[:, ct, s-tile].
  C) attention as ONE flattened pair stream across chunks: pst [128,2,512]
     psum (fp8 DoubleRow), one exp per pair (Act), tri-mask on diag (DVE),
     po[66,512] += v_aug^T wst; PE-transpose po; out = cols / col64.
     pst(p+1) is always emitted before po(p); proj/tail units are interleaved
     as deadline-scheduled fillers so PE never idles while Act runs exp.
"""

import numpy as np
import ml_dtypes

import concourse.bacc as bacc
import concourse.mybir as mybir
import concourse.tile as tile
from concourse.bass_utils import run_bass_kernel_spmd

F32 = mybir.dt.float32
F32R = mybir.dt.float32r
BF16 = mybir.dt.bfloat16
FP8 = mybir.dt.float8e4

B, T, C, H = 8, 2048, 1024, 64
NCT = C // 128          # 8 c-tiles
NCH = T // 512          # 4 t-chunks
SCALE = float(C ** -0.5)

USE_FP8 = True
N_WARM = 27             # PE warmup matmuls (cover DMA startup, beat pstate ramp)

_CACHE = {}
EMIT = {}


def build(fp8=USE_FP8, n_warm=N_WARM, dma_split=False):
    EMIT.clear()
    EMIT.update({"PE": [], "Act": [], "DVE": [], "Pool": []})
    pe = EMIT["PE"].append
    dv = EMIT["DVE"].append
    ac = EMIT["Act"].append
    pl = EMIT["Pool"].append
    nc = bacc.Bacc(name="head_attn3")
    x_d = nc.dram_tensor("xb", [T, C], BF16, kind="ExternalInput")
    w_d = nc.dram_tensor("wqkv", [NCT * 192, 128], BF16, kind="ExternalInput")
    out_d = nc.dram_tensor("out", [T, H], F32, kind="ExternalOutput")
    out_r = out_d.rearrange("(a p) h -> p a h", p=128)

    with tile.TileContext(nc) as tc:
        with (
            tc.tile_pool(name="singles", bufs=1) as singles,
            tc.tile_pool(name="wstp", bufs=3) as wstp,
            tc.tile_pool(name="outp", bufs=2) as outp,
            tc.tile_pool(name="ppst", bufs=2, space="PSUM") as ppst,
            tc.tile_pool(name="pproj", bufs=2, space="PSUM") as pproj,
            tc.tile_pool(name="pacc", bufs=2, space="PSUM") as pacc,
        ):
            # --- Act exp-table warmup + PE pstate warmup (independent tiles)
            warm = singles.tile([128, 256], BF16)
            warma = singles.tile([128, 8], BF16)
            nc.gpsimd.memset(warm, 0.0)
            ac("warma")
            nc.scalar.activation(warma, warma,
                                 mybir.ActivationFunctionType.Exp)
            warmp = pproj.tile([128, 256], F32, tag="proj", name="warmp")
            for wi in range(n_warm):
                pe(f"warm{wi}")
                nc.tensor.matmul(warmp, warm[:, 0:128], warm,
                                 start=True, stop=True)

            # --- weights loaded via XBAR transpose like x (uniform DMA type
            # on SP avoids tripping the queue convoy)
            wqkv = singles.tile([128, NCT, 192], BF16)
            nc.sync.dma_start(
                wqkv.rearrange("p a b -> p (a b)"), w_d[:, :], transpose=True)

            # --- identity [66,66] f32r generated on-device (no DMA)
            identF = singles.tile([66, 66], F32)
            nc.gpsimd.memset(identF, 1.0)
            nc.gpsimd.affine_select(identF, identF, pattern=[[1, 66]],
                                    compare_op=mybir.AluOpType.is_equal,
                                    fill=0.0, base=0, channel_multiplier=-1)
            identR = singles.tile([66, 66], F32R)
            dv("identR")
            nc.vector.tensor_copy(identR, identF)

            # --- x transposed loads (XBAR), pure stream on SP
            xT = singles.tile([128, NCT, T], BF16)
            spans = [(0, 512), (512, 1024), (1024, 2048)]
            for si, (t0, t1) in enumerate(spans):
                for ct in range(NCT):
                    nc.sync.dma_start(
                        xT[:, ct, t0:t1],
                        x_d[t0:t1, ct * 128:(ct + 1) * 128],
                        transpose=True)

            # --- q/k stores (base 0; k copy partition-shifted 64->0)
            if fp8:
                qS = singles.tile([64, 2, T], FP8)
                kS = singles.tile([64, 2, T], FP8)
                nc.gpsimd.memset(qS[:, 1, :], 0.0)
                nc.gpsimd.memset(kS[:, 1, :], 0.0)
            else:
                qS = singles.tile([64, 1, T], BF16)
                kS = singles.tile([64, 1, T], BF16)

            v_aug = singles.tile([128, T // 128, 66], BF16)
            nc.gpsimd.memset(v_aug[:, :, 64:66], 1.0)

            out_sb = singles.tile([128, T // 128, H], F32)

            qk_cur = [None]

            def proj_qk(i, part):
                if part == 0:
                    qk_cur[0] = pproj.tile([128, 512], F32, tag="proj", name="pqk")
                pqk = qk_cur[0]
                for ct in range(4 * part, 4 * part + 4):
                    pe(f"qk{i}.{ct}")
                    nc.tensor.matmul(pqk, wqkv[:, ct, 0:128],
                                     xT[:, ct, i * 512:(i + 1) * 512],
                                     start=(ct == 0), stop=(ct == NCT - 1))
                if part == 1:
                    cs = slice(i * 512, (i + 1) * 512)
                    dv(f"qScp{i}")
                    nc.vector.tensor_copy(qS[:, 0, cs], pqk[0:64, :])
                    dv(f"kScp{i}")
                    nc.vector.tensor_copy(kS[:, 0, cs], pqk[64:128, :])

            def proj_v(s):
                pv = pproj.tile([128, H], F32, tag="proj", name="pv")
                for ct in range(NCT):
                    pe(f"v{s}.{ct}")
                    nc.tensor.matmul(pv, xT[:, ct, s * 128:(s + 1) * 128],
                                     wqkv[:, ct, 128:192],
                                     start=(ct == 0), stop=(ct == NCT - 1))
                dv(f"vcp{s}")
                nc.vector.tensor_copy(v_aug[:, s, 0:H], pv)

            po_tiles = {}
            pn_tiles = {}

            def pair_ds(i, m):
                ds = []
                for u in range(2):
                    j = 2 * m + u
                    kk = j - 4 * i
                    ds.append((j, kk, 128 * kk if kk > 0 else 0))
                return ds

            def pst_exp(i, m):
                ds = pair_ds(i, m)
                pst = ppst.tile([128, 2, 512], F32, tag="pst", name="pst")
                for u, (j, kk, d) in enumerate(ds):
                    pe(f"pst{i}.{m}.{u}")
                    if fp8:
                        nc.tensor.matmul(
                            pst[:, u, d:], kS[:, :, j * 128:(j + 1) * 128],
                            qS[:, :, i * 512 + d:(i + 1) * 512],
                            start=True, stop=True,
                            perf_mode=mybir.MatmulPerfMode.DoubleRow)
                    else:
                        nc.tensor.matmul(
                            pst[:, u, d:], kS[:, 0, j * 128:(j + 1) * 128],
                            qS[:, 0, i * 512 + d:(i + 1) * 512],
                            start=True, stop=True)
                wst = wstp.tile([128, 2, 512], BF16, tag="wst", name="wst")
                dp = ds[0][2]
                ac(f"exp{i}.{m}")
                nc.scalar.activation(wst[:, :, dp:], pst[:, :, dp:],
                                     mybir.ActivationFunctionType.Exp,
                                     scale=SCALE)
                for u, (j, kk, d) in enumerate(ds):
                    if kk >= 0:
                        pl(f"mask{i}.{m}.{u}")
                        nc.gpsimd.affine_select(
                            wst[:, u, d:d + 128], wst[:, u, d:d + 128],
                            pattern=[[1, 128]],
                            compare_op=mybir.AluOpType.is_ge,
                            fill=0.0, base=0, channel_multiplier=-1)
                return wst

            def po_pair(i, m, wst):
                if m == 0:
                    po_tiles[i] = pacc.tile([66, 512], F32, tag="acc", name="po")
                po = po_tiles[i]
                nj = 4 * i + 4
                for u, (j, kk, d) in enumerate(pair_ds(i, m)):
                    pe(f"po{i}.{m}.{u}")
                    nc.tensor.matmul(po[:, d:], v_aug[:, j, 0:66],
                                     wst[:, u, d:],
                                     start=(j == 0), stop=(j == nj - 1))

            def tail(i):
                po = po_tiles[i]
                oT = outp.tile([66, 512], F32R, tag="oT", name="oT")
                dv(f"oTcp{i}")
                nc.vector.tensor_copy(oT, po)
                pn = pacc.tile([128, 4, 66], F32R, tag="acc", name="pn")
                pn_tiles[i] = pn
                for b in range(4):
                    pe(f"tr{i}.{b}")
                    nc.tensor.transpose(pn[:, b, :],
                                        oT[:, b * 128:(b + 1) * 128],
                                        identR[0:66, 0:66])
                rec = outp.tile([128, 4], F32, tag="rec", name="rec")
                dv(f"rec{i}")
                nc.vector.reciprocal(rec, pn[:, :, 64:65])
                for b in range(4):
                    dv(f"mul{i}.{b}")
                    nc.vector.tensor_scalar_mul(out_sb[:, 4 * i + b, :],
                                                pn[:, b, 0:64], rec[:, b:b + 1])
                nc.sync.dma_start(out_r[:, 4 * i:4 * i + 4, :],
                                  out_sb[:, 4 * i:4 * i + 4, :])

            # --- flattened pair stream with deadline-scheduled fillers -----
            # filler entries: (deadline, fn); deadline = ('pst', i) emitted
            # before pst(i, 0); ('po', i, m) before po(i, m).
            fq = []
            for i in range(NCH):
                if i + 1 < NCH:
                    # next-chunk qk proj first: feeds Act across the boundary
                    fq.append((('pst', i + 1), lambda i=i: proj_qk(i + 1, 0)))
                    fq.append((('pst', i + 1), lambda i=i: proj_qk(i + 1, 1)))
                if i > 0:
                    fq.append((('po', i, 0), lambda i=i: tail(i - 1)))
                lo = 2 if i == 0 else 0
                for b in range(lo, 4):
                    s = 4 * i + b
                    fq.append((('po', i, s // 2), lambda s=s: proj_v(s)))

            def dl_le(dl, bound):
                # compare deadlines: ('pst', i) sorts before ('po', i, m)
                key = {'pst': 0, 'po': 1}
                a = (dl[1], key[dl[0]], dl[2] if len(dl) > 2 else -1)
                b = (bound[1], key[bound[0]], bound[2] if len(bound) > 2 else -1)
                return a <= b

            def flush(bound):
                # scan whole queue: deadlines are not monotonic in queue order
                rest = []
                for ent in fq:
                    if dl_le(ent[0], bound):
                        ent[1]()
                    else:
                        rest.append(ent)
                fq[:] = rest

            proj_qk(0, 0)
            proj_qk(0, 1)
            proj_v(0)
            proj_v(1)

            all_pairs = [(i, m) for i in range(NCH) for m in range(2 * i + 2)]
            prev = None
            for (i, m) in all_pairs:
                if m == 0:
                    flush(('pst', i))
                wst = pst_exp(i, m)
                if prev is not None:
                    flush(('po', prev[0], prev[1]))
                    po_pair(*prev)
                if fq:
                    fq.pop(0)[1]()
                prev = (i, m, wst)
            flush(('po', NCH - 1, 10 ** 6))
            po_pair(*prev)
            tail(NCH - 1)

    nc.compile()
    return nc


def _host_prep(x, Wq, Wk, Wv):
    bf = ml_dtypes.bfloat16
    xb = np.ascontiguousarray(x).astype(bf)
    w = np.concatenate([Wq, Wk, Wv], axis=1)          # [1024, 192]
    wqkv = np.ascontiguousarray(
        w.reshape(NCT, 128, 192).transpose(1, 0, 2).reshape(128, NCT * 192).T
    ).astype(bf)
    return xb, wqkv


def kernel(x, Wq, Wk, Wv, trace=False):
    x = np.asarray(x, dtype=np.float32)
    Wq = np.asarray(Wq, dtype=np.float32)
    Wk = np.asarray(Wk, dtype=np.float32)
    Wv = np.asarray(Wv, dtype=np.float32)

    if "nc" not in _CACHE:
        _CACHE["nc"] = build()
    nc = _CACHE["nc"]

    xb, wqkv = _host_prep(x, Wq, Wk, Wv)
    in_maps = [{"xb": xb[b], "wqkv": wqkv} for b in range(B)]
    try:
        res = run_bass_kernel_spmd(nc, in_maps, core_ids=list(range(B)), trace=trace)
    except ModuleNotFoundError:
        res = run_bass_kernel_spmd(nc, in_maps, core_ids=list(range(B)))
    out = np.stack([r["out"] for r in res.results], axis=0)
    kernel.last_exec_time_ns = res.exec_time_ns
    kernel.last_results = res
    return out


# revision 3
# speedup vs baseline: 1.0462x; 1.0436x over previous
"""Single-head causal attention on 8 TRN2 NeuronCores, batch-parallel (v3).

Problem: x[8,2048,1024] f32, Wq/Wk/Wv[1024,64] f32
  q,k,v = x@W*  ;  scores = q k^T / sqrt(1024), causal  ;  out = softmax(scores) @ v

Sharding: batch dim across 8 cores (1 batch element per core, no collectives).
Host prep: cast to bf16; weights packed [Wq|Wk|Wv] -> [128, 8, 192].

Per-core dataflow:
  A) x bf16 loaded TRANSPOSED via XBAR DMA transpose (t-quarters then a
     half, one pure stream on the SP queue; weights too) -> xT [128x8ct, t].
  B) per t-chunk (512): packed qk proj (M=128) -> psum; DVE copies rows 0:64
     -> qS, rows 64:128 -> kS (partition-shifted), fp8e4 (zero 2nd DoubleRow
     k-tile) or bf16. v proj DIRECT in [s,h]: lhsT = xT[:, ct, s-tile].
  C) attention as ONE flattened pair stream across chunks: pst [128,2,512]
     psum (fp8 DoubleRow), one exp per pair (Act), tri-mask on diag (DVE),
     po[66,512] += v_aug^T wst; PE-transpose po; out = cols / col64.
     pst(p+1) is always emitted before po(p); proj/tail units are interleaved
     as deadline-scheduled fillers so PE never idles while Act runs exp.
"""

import numpy as np
import ml_dtypes

import concourse.bacc as bacc
import concourse.mybir as mybir
import concourse.tile as tile
from concourse.bass_utils import run_bass_kernel_spmd

F32 = mybir.dt.float32
F32R = mybir.dt.float32r
BF16 = mybir.dt.bfloat16
FP8 = mybir.dt.float8e4

B, T, C, H = 8, 2048, 1024, 64
NCT = C // 128          # 8 c-tiles
NCH = T // 512          # 4 t-chunks
SCALE = float(C ** -0.5)

USE_FP8 = True
N_WARM = 27             # PE warmup matmuls (cover DMA startup, beat pstate ramp)

_CACHE = {}
EMIT = {}


def build(fp8=USE_FP8, n_warm=N_WARM, dma_split=False):
    EMIT.clear()
    EMIT.update({"PE": [], "Act": [], "DVE": [], "Pool": []})
    pe = EMIT["PE"].append
    dv = EMIT["DVE"].append
    ac = EMIT["Act"].append
    pl = EMIT["Pool"].append
    nc = bacc.Bacc(name="head_attn3")
    x_d = nc.dram_tensor("xb", [T, C], BF16, kind="ExternalInput")
    w_d = nc.dram_tensor("wqkv", [NCT * 192, 128], BF16, kind="ExternalInput")
    out_d = nc.dram_tensor("out", [T, H], F32, kind="ExternalOutput")
    out_r = out_d.rearrange("(a p) h -> p a h", p=128)

    with tile.TileContext(nc) as tc:
        with (
            tc.tile_pool(name="singles", bufs=1) as singles,
            tc.tile_pool(name="wstp", bufs=4) as wstp,
            tc.tile_pool(name="outp", bufs=3) as outp,
            tc.tile_pool(name="ppst", bufs=2, space="PSUM") as ppst,
            tc.tile_pool(name="pproj", bufs=2, space="PSUM") as pproj,
            tc.tile_pool(name="pacc", bufs=2, space="PSUM") as pacc,
        ):
            # --- Act exp-table warmup + PE pstate warmup (independent tiles)
            warm = singles.tile([128, 256], BF16)
            warma = singles.tile([128, 8], BF16)
            nc.gpsimd.memset(warm, 0.0)
            ac("warma")
            nc.scalar.activation(warma, warma,
                                 mybir.ActivationFunctionType.Exp)
            warmp = pproj.tile([128, 256], F32, tag="proj", name="warmp")
            for wi in range(n_warm):
                pe(f"warm{wi}")
                nc.tensor.matmul(warmp, warm[:, 0:128], warm,
                                 start=True, stop=True)

            # --- weights loaded via XBAR transpose like x (uniform DMA type
            # on SP avoids tripping the queue convoy)
            wqkv = singles.tile([128, NCT, 192], BF16)
            nc.sync.dma_start(
                wqkv.rearrange("p a b -> p (a b)"), w_d[:, :], transpose=True)

            # --- identity [66,66] f32r generated on-device (no DMA)
            identF = singles.tile([66, 66], F32)
            nc.gpsimd.memset(identF, 1.0)
            nc.gpsimd.affine_select(identF, identF, pattern=[[1, 66]],
                                    compare_op=mybir.AluOpType.is_equal,
                                    fill=0.0, base=0, channel_multiplier=-1)
            identR = singles.tile([66, 66], F32R)
            dv("identR")
            nc.vector.tensor_copy(identR, identF)

            # --- x transposed loads (XBAR), pure stream on SP
            xT = singles.tile([128, NCT, T], BF16)
            spans = [(0, 512), (512, 1024), (1024, 2048)]
            for si, (t0, t1) in enumerate(spans):
                for ct in range(NCT):
                    nc.sync.dma_start(
                        xT[:, ct, t0:t1],
                        x_d[t0:t1, ct * 128:(ct + 1) * 128],
                        transpose=True)

            # --- q/k stores (base 0; k copy partition-shifted 64->0)
            if fp8:
                qS = singles.tile([64, 2, T], FP8)
                kS = singles.tile([64, 2, T], FP8)
                nc.gpsimd.memset(qS[:, 1, :], 0.0)
                nc.gpsimd.memset(kS[:, 1, :], 0.0)
            else:
                qS = singles.tile([64, 1, T], BF16)
                kS = singles.tile([64, 1, T], BF16)

            v_aug = singles.tile([128, T // 128, 66], BF16)
            nc.gpsimd.memset(v_aug[:, :, 64:66], 1.0)

            out_sb = singles.tile([128, T // 128, H], F32)

            qk_cur = [None]

            def proj_qk(i, part):
                if part == 0:
                    qk_cur[0] = pproj.tile([128, 512], F32, tag="proj", name="pqk")
                pqk = qk_cur[0]
                for ct in range(4 * part, 4 * part + 4):
                    pe(f"qk{i}.{ct}")
                    nc.tensor.matmul(pqk, wqkv[:, ct, 0:128],
                                     xT[:, ct, i * 512:(i + 1) * 512],
                                     start=(ct == 0), stop=(ct == NCT - 1))
                if part == 1:
                    cs = slice(i * 512, (i + 1) * 512)
                    dv(f"qScp{i}")
                    nc.vector.tensor_copy(qS[:, 0, cs], pqk[0:64, :])
                    dv(f"kScp{i}")
                    nc.vector.tensor_copy(kS[:, 0, cs], pqk[64:128, :])

            def proj_v(s):
                pv = pproj.tile([128, H], F32, tag="proj", name="pv")
                for ct in range(NCT):
                    pe(f"v{s}.{ct}")
                    nc.tensor.matmul(pv, xT[:, ct, s * 128:(s + 1) * 128],
                                     wqkv[:, ct, 128:192],
                                     start=(ct == 0), stop=(ct == NCT - 1))
                dv(f"vcp{s}")
                nc.vector.tensor_copy(v_aug[:, s, 0:H], pv)

            po_tiles = {}
            pn_tiles = {}

            def pair_ds(i, m):
                ds = []
                for u in range(2):
                    j = 2 * m + u
                    kk = j - 4 * i
                    ds.append((j, kk, 128 * kk if kk > 0 else 0))
                return ds

            def pst_exp(i, m):
                ds = pair_ds(i, m)
                pst = ppst.tile([128, 2, 512], F32, tag="pst", name="pst")
                for u, (j, kk, d) in enumerate(ds):
                    pe(f"pst{i}.{m}.{u}")
                    if fp8:
                        nc.tensor.matmul(
                            pst[:, u, d:], kS[:, :, j * 128:(j + 1) * 128],
                            qS[:, :, i * 512 + d:(i + 1) * 512],
                            start=True, stop=True,
                            perf_mode=mybir.MatmulPerfMode.DoubleRow)
                    else:
                        nc.tensor.matmul(
                            pst[:, u, d:], kS[:, 0, j * 128:(j + 1) * 128],
                            qS[:, 0, i * 512 + d:(i + 1) * 512],
                            start=True, stop=True)
                wst = wstp.tile([128, 2, 512], BF16, tag="wst", name="wst")
                dp = ds[0][2]
                ac(f"exp{i}.{m}")
                nc.scalar.activation(wst[:, :, dp:], pst[:, :, dp:],
                                     mybir.ActivationFunctionType.Exp,
                                     scale=SCALE)
                for u, (j, kk, d) in enumerate(ds):
                    if kk >= 0:
                        pl(f"mask{i}.{m}.{u}")
                        nc.gpsimd.affine_select(
                            wst[:, u, d:d + 128], wst[:, u, d:d + 128],
                            pattern=[[1, 128]],
                            compare_op=mybir.AluOpType.is_ge,
                            fill=0.0, base=0, channel_multiplier=-1)
                return wst

            def po_pair(i, m, wst):
                if m == 0:
                    po_tiles[i] = pacc.tile([66, 512], F32, tag="acc", name="po")
                po = po_tiles[i]
                nj = 4 * i + 4
                for u, (j, kk, d) in enumerate(pair_ds(i, m)):
                    pe(f"po{i}.{m}.{u}")
                    nc.tensor.matmul(po[:, d:], v_aug[:, j, 0:66],
                                     wst[:, u, d:],
                                     start=(j == 0), stop=(j == nj - 1))

            def tail(i):
                po = po_tiles[i]
                oT = outp.tile([66, 512], F32R, tag="oT", name="oT")
                dv(f"oTcp{i}")
                nc.vector.tensor_copy(oT, po)
                pn = pacc.tile([128, 4, 66], F32R, tag="acc", name="pn")
                pn_tiles[i] = pn
                for b in range(4):
                    pe(f"tr{i}.{b}")
                    nc.tensor.transpose(pn[:, b, :],
                                        oT[:, b * 128:(b + 1) * 128],
                                        identR[0:66, 0:66])
                rec = outp.tile([128, 4], F32, tag="rec", name="rec")
                dv(f"rec{i}")
                nc.vector.reciprocal(rec, pn[:, :, 64:65])
                for b in range(4):
                    dv(f"mul{i}.{b}")
                    nc.vector.tensor_scalar_mul(out_sb[:, 4 * i + b, :],
                                                pn[:, b, 0:64], rec[:, b:b + 1])
                nc.sync.dma_start(out_r[:, 4 * i:4 * i + 4, :],
                                  out_sb[:, 4 * i:4 * i + 4, :])

            # --- flattened pair stream with deadline-scheduled fillers -----
            # filler entries: (deadline, fn); deadline = ('pst', i) emitted
            # before pst(i, 0); ('po', i, m) before po(i, m).
            fq = []
            for i in range(NCH):
                if i + 1 < NCH:
                    # next-chunk qk proj first: feeds Act across the boundary
                    fq.append((('pst', i + 1), lambda i=i: proj_qk(i + 1, 0)))
                    fq.append((('pst', i + 1), lambda i=i: proj_qk(i + 1, 1)))
                if i > 0:
                    fq.append((('po', i, 0), lambda i=i: tail(i - 1)))
                lo = 2 if i == 0 else 0
                for b in range(lo, 4):
                    s = 4 * i + b
                    fq.append((('po', i, s // 2), lambda s=s: proj_v(s)))

            def dl_le(dl, bound):
                # compare deadlines: ('pst', i) sorts before ('po', i, m)
                key = {'pst': 0, 'po': 1}
                a = (dl[1], key[dl[0]], dl[2] if len(dl) > 2 else -1)
                b = (bound[1], key[bound[0]], bound[2] if len(bound) > 2 else -1)
                return a <= b

            def flush(bound):
                # scan whole queue: deadlines are not monotonic in queue order
                rest = []
                for ent in fq:
                    if dl_le(ent[0], bound):
                        ent[1]()
                    else:
                        rest.append(ent)
                fq[:] = rest

            proj_qk(0, 0)
            proj_qk(0, 1)
            proj_v(0)
            proj_v(1)

            all_pairs = [(i, m) for i in range(NCH) for m in range(2 * i + 2)]
            prev = None
            for (i, m) in all_pairs:
                if m == 0:
                    flush(('pst', i))
                wst = pst_exp(i, m)
                if prev is not None:
                    flush(('po', prev[0], prev[1]))
                    po_pair(*prev)
                if fq:
                    fq.pop(0)[1]()
                prev = (i, m, wst)
            flush(('po', NCH - 1, 10 ** 6))
            po_pair(*prev)
            tail(NCH - 1)

    nc.compile()
    return nc


def _host_prep(x, Wq, Wk, Wv):
    bf = ml_dtypes.bfloat16
    xb = np.ascontiguousarray(x).astype(bf)
    w = np.concatenate([Wq, Wk, Wv], axis=1)          # [1024, 192]
    wqkv = np.ascontiguousarray(
        w.reshape(NCT, 128, 192).transpose(1, 0, 2).reshape(128, NCT * 192).T
    ).astype(bf)
    return xb, wqkv


def kernel(x, Wq, Wk, Wv, trace=False):
    x = np.asarray(x, dtype=np.float32)
    Wq = np.asarray(Wq, dtype=np.float32)
    Wk = np.asarray(Wk, dtype=np.float32)
    Wv = np.asarray(Wv, dtype=np.float32)

    if "nc" not in _CACHE:
        _CACHE["nc"] = build()
    nc = _CACHE["nc"]

    xb, wqkv = _host_prep(x, Wq, Wk, Wv)
    in_maps = [{"xb": xb[b], "wqkv": wqkv} for b in range(B)]
    try:
        res = run_bass_kernel_spmd(nc, in_maps, core_ids=list(range(B)), trace=trace)
    except ModuleNotFoundError:
        res = run_bass_kernel_spmd(nc, in_maps, core_ids=list(range(B)))
    out = np.stack([r["out"] for r in res.results], axis=0)
    kernel.last_exec_time_ns = res.exec_time_ns
    kernel.last_results = res
    return out


# revision 5
# speedup vs baseline: 1.0641x; 1.0172x over previous
"""Single-head causal attention on 8 TRN2 NeuronCores, batch-parallel (v3).

Problem: x[8,2048,1024] f32, Wq/Wk/Wv[1024,64] f32
  q,k,v = x@W*  ;  scores = q k^T / sqrt(1024), causal  ;  out = softmax(scores) @ v

Sharding: batch dim across 8 cores (1 batch element per core, no collectives).
Host prep: cast to bf16; weights packed [Wq|Wk|Wv] -> [128, 8, 192].

Per-core dataflow:
  A) x bf16 loaded TRANSPOSED via XBAR DMA transpose (t-quarters then halves,
     split across SP/Act issue queues) -> xT [c=128 x 8ct, t=2048].
  B) per t-chunk (512): packed qk proj (M=128) -> psum; DVE copies rows 0:64
     -> qS, rows 64:128 -> kS (partition-shifted), fp8e4 (zero 2nd DoubleRow
     k-tile) or bf16. v proj DIRECT in [s,h]: lhsT = xT[:, ct, s-tile].
  C) attention as ONE flattened pair stream across chunks: pst [128,2,512]
     psum (fp8 DoubleRow), one exp per pair (Act), tri-mask on diag (DVE),
     po[66,512] += v_aug^T wst; PE-transpose po; out = cols / col64.
     pst(p+1) is always emitted before po(p); proj/tail units are interleaved
     as deadline-scheduled fillers so PE never idles while Act runs exp.
"""

import numpy as np
import ml_dtypes

import concourse.bacc as bacc
import concourse.mybir as mybir
import concourse.tile as tile
from concourse.bass_utils import run_bass_kernel_spmd

F32 = mybir.dt.float32
F32R = mybir.dt.float32r
BF16 = mybir.dt.bfloat16
FP8 = mybir.dt.float8e4

B, T, C, H = 8, 2048, 1024, 64
NCT = C // 128          # 8 c-tiles
NCH = T // 512          # 4 t-chunks
SCALE = float(C ** -0.5)

USE_FP8 = True
N_WARM = 25             # PE warmup matmuls (cover DMA startup, beat pstate ramp)

_CACHE = {}
EMIT = {}


def build(fp8=USE_FP8, n_warm=N_WARM, dma_split=False):
    EMIT.clear()
    EMIT.update({"PE": [], "Act": [], "DVE": [], "Pool": []})
    pe = EMIT["PE"].append
    dv = EMIT["DVE"].append
    ac = EMIT["Act"].append
    pl = EMIT["Pool"].append
    nc = bacc.Bacc(name="head_attn3")
    x_d = nc.dram_tensor("xb", [T, C], BF16, kind="ExternalInput")
    w_d = nc.dram_tensor("wqkv", [NCT * 192, 128], BF16, kind="ExternalInput")
    out_d = nc.dram_tensor("out", [T, H], BF16, kind="ExternalOutput")
    out_r = out_d.rearrange("(a p) h -> p a h", p=128)

    with tile.TileContext(nc) as tc:
        with (
            tc.tile_pool(name="singles", bufs=1) as singles,
            tc.tile_pool(name="wstp", bufs=4) as wstp,
            tc.tile_pool(name="outp", bufs=3) as outp,
            tc.tile_pool(name="ppst", bufs=2, space="PSUM") as ppst,
            tc.tile_pool(name="pproj", bufs=2, space="PSUM") as pproj,
            tc.tile_pool(name="pacc", bufs=2, space="PSUM") as pacc,
        ):
            # --- Act exp-table warmup + PE pstate warmup (independent tiles)
            warm = singles.tile([128, 256], BF16)
            warma = singles.tile([128, 8], BF16)
            nc.gpsimd.memset(warm, 0.0)
            ac("warma")
            nc.scalar.activation(warma, warma,
                                 mybir.ActivationFunctionType.Exp)
            warmp = pproj.tile([128, 256], F32, tag="proj", name="warmp")
            for wi in range(n_warm):
                pe(f"warm{wi}")
                nc.tensor.matmul(warmp, warm[:, 0:128], warm,
                                 start=True, stop=True)

            # --- weights loaded via XBAR transpose like x (uniform DMA type
            # on SP avoids tripping the queue convoy)
            wqkv = singles.tile([128, NCT, 192], BF16)
            nc.sync.dma_start(
                wqkv.rearrange("p a b -> p (a b)"), w_d[:, :], transpose=True)

            # --- identity [66,66] f32r generated on-device (no DMA)
            triB = singles.tile([128, 128], BF16)
            nc.gpsimd.memset(triB, 1.0)
            nc.gpsimd.affine_select(triB, triB, pattern=[[1, 128]],
                                    compare_op=mybir.AluOpType.is_ge,
                                    fill=0.0, base=0, channel_multiplier=-1)

            identF = singles.tile([66, 66], F32)
            nc.gpsimd.memset(identF, 1.0)
            nc.gpsimd.affine_select(identF, identF, pattern=[[1, 66]],
                                    compare_op=mybir.AluOpType.is_equal,
                                    fill=0.0, base=0, channel_multiplier=-1)
            identR = singles.tile([66, 66], F32R)
            dv("identR")
            nc.vector.tensor_copy(identR, identF)

            # --- x transposed loads (XBAR), pure stream on SP
            xT = singles.tile([128, NCT, T], BF16)
            spans = [(0, 512), (512, 1024), (1024, 2048)]
            for si, (t0, t1) in enumerate(spans):
                for ct in range(NCT):
                    nc.sync.dma_start(
                        xT[:, ct, t0:t1],
                        x_d[t0:t1, ct * 128:(ct + 1) * 128],
                        transpose=True)

            # --- q/k stores (base 0; k copy partition-shifted 64->0)
            if fp8:
                qSf = singles.tile([128, 2, T], FP8)
                kS = singles.tile([64, 2, T], FP8)
                nc.gpsimd.memset(qSf[:, 1, :], 0.0)
                nc.gpsimd.memset(kS[:, 1, :], 0.0)
            else:
                qSf = singles.tile([128, 1, T], BF16)
                kS = singles.tile([64, 1, T], BF16)
            qS = qSf[0:64]

            v_aug = singles.tile([128, T // 128, 66], BF16)
            nc.gpsimd.memset(v_aug[:, :, 64:66], 1.0)

            out_sb = singles.tile([128, T // 128, H], BF16)

            qk_cur = [None]

            def proj_qk(i, part):
                if part == 0:
                    qk_cur[0] = pproj.tile([128, 512], F32, tag="proj", name="pqk")
                pqk = qk_cur[0]
                for ct in range(4 * part, 4 * part + 4):
                    pe(f"qk{i}.{ct}")
                    nc.tensor.matmul(pqk, wqkv[:, ct, 0:128],
                                     xT[:, ct, i * 512:(i + 1) * 512],
                                     start=(ct == 0), stop=(ct == NCT - 1))
                if part == 1:
                    cs = slice(i * 512, (i + 1) * 512)
                    dv(f"qScp{i}")
                    nc.vector.tensor_copy(qSf[:, 0, cs], pqk)
                    if i <= 1:
                        # Act is idle before the first exp; run the k copy
                        # there so q/k copies overlap on the startup chain
                        ac(f"kScp{i}")
                        nc.scalar.copy(kS[:, 0, cs], pqk[64:128, :])
                    else:
                        dv(f"kScp{i}")
                        nc.vector.tensor_copy(kS[:, 0, cs], pqk[64:128, :])

            def proj_v(s):
                pv = pproj.tile([128, H], F32, tag="proj", name="pv")
                for ct in range(NCT):
                    pe(f"v{s}.{ct}")
                    nc.tensor.matmul(pv, xT[:, ct, s * 128:(s + 1) * 128],
                                     wqkv[:, ct, 128:192],
                                     start=(ct == 0), stop=(ct == NCT - 1))
                dv(f"vcp{s}")
                nc.vector.tensor_copy(v_aug[:, s, 0:H], pv)

            po_tiles = {}
            pn_tiles = {}

            def pair_ds(i, m):
                ds = []
                for u in range(2):
                    j = 2 * m + u
                    kk = j - 4 * i
                    ds.append((j, kk, 128 * kk if kk > 0 else 0))
                return ds

            def pst_exp(i, m):
                ds = pair_ds(i, m)
                pst = ppst.tile([128, 2, 512], F32, tag="pst", name="pst")
                for u, (j, kk, d) in enumerate(ds):
                    pe(f"pst{i}.{m}.{u}")
                    if fp8:
                        nc.tensor.matmul(
                            pst[:, u, d:], kS[:, :, j * 128:(j + 1) * 128],
                            qS[:, :, i * 512 + d:(i + 1) * 512],
                            start=True, stop=True,
                            perf_mode=mybir.MatmulPerfMode.DoubleRow)
                    else:
                        nc.tensor.matmul(
                            pst[:, u, d:], kS[:, 0, j * 128:(j + 1) * 128],
                            qS[:, 0, i * 512 + d:(i + 1) * 512],
                            start=True, stop=True)
                wst = wstp.tile([128, 2, 512], BF16, tag="wst", name="wst")
                dp = ds[0][2]
                ac(f"exp{i}.{m}")
                nc.scalar.activation(wst[:, :, dp:], pst[:, :, dp:],
                                     mybir.ActivationFunctionType.Exp,
                                     scale=SCALE)
                for u, (j, kk, d) in enumerate(ds):
                    if kk >= 0:
                        if True:
                            dv(f"mask{i}.{m}.{u}")
                            nc.vector.tensor_mul(wst[:, u, d:d + 128],
                                                 wst[:, u, d:d + 128], triB)
                        else:
                            pl(f"mask{i}.{m}.{u}")
                            nc.gpsimd.affine_select(
                                wst[:, u, d:d + 128], wst[:, u, d:d + 128],
                                pattern=[[1, 128]],
                                compare_op=mybir.AluOpType.is_ge,
                                fill=0.0, base=0, channel_multiplier=-1)
                return wst

            def po_pair(i, m, wst):
                if m == 0:
                    po_tiles[i] = pacc.tile([66, 512], F32, tag="acc", name="po")
                po = po_tiles[i]
                nj = 4 * i + 4
                for u, (j, kk, d) in enumerate(pair_ds(i, m)):
                    pe(f"po{i}.{m}.{u}")
                    nc.tensor.matmul(po[:, d:], v_aug[:, j, 0:66],
                                     wst[:, u, d:],
                                     start=(j == 0), stop=(j == nj - 1))

            def tail(i):
                po = po_tiles[i]
                oT = outp.tile([66, 512], F32R, tag="oT", name="oT")
                dv(f"oTcp{i}")
                nc.vector.tensor_copy(oT, po)
                pn = pacc.tile([128, 4, 66], F32R, tag="acc", name="pn")
                pn_tiles[i] = pn
                for b in range(4):
                    pe(f"tr{i}.{b}")
                    nc.tensor.transpose(pn[:, b, :],
                                        oT[:, b * 128:(b + 1) * 128],
                                        identR[0:66, 0:66])
                rec = outp.tile([128, 4, 1], F32, tag="rec", name="rec")
                dv(f"rec{i}")
                nc.vector.reciprocal(rec, pn[:, :, 64:65])
                dv(f"mul{i}")
                nc.vector.tensor_mul(out_sb[:, 4 * i:4 * i + 4, :],
                                     pn[:, :, 0:64],
                                     rec.broadcast_to([128, 4, H]))
                nc.sync.dma_start(out_r[:, 4 * i:4 * i + 4, :],
                                  out_sb[:, 4 * i:4 * i + 4, :])

            # --- flattened pair stream with deadline-scheduled fillers -----
            # filler entries: (deadline, fn); deadline = ('pst', i) emitted
            # before pst(i, 0); ('po', i, m) before po(i, m).
            fq = []
            for i in range(NCH):
                def vf(b, i=i):
                    s = 4 * i + b
                    fq.append((('po', i, s // 2), lambda s=s: proj_v(s)))
                if i + 1 < NCH:
                    fq.append((('pst', i + 1), lambda i=i: proj_qk(i + 1, 0)))
                    fq.append((('pst', i + 1), lambda i=i: proj_qk(i + 1, 1)))
                if i > 0:
                    fq.append((('po', i, 0), lambda i=i: tail(i - 1)))
                    vf(0), vf(1)
                vf(2), vf(3)

            def dl_le(dl, bound):
                # compare deadlines: ('pst', i) sorts before ('po', i, m)
                key = {'pst': 0, 'po': 1}
                a = (dl[1], key[dl[0]], dl[2] if len(dl) > 2 else -1)
                b = (bound[1], key[bound[0]], bound[2] if len(bound) > 2 else -1)
                return a <= b

            def flush(bound):
                # scan whole queue: deadlines are not monotonic in queue order
                rest = []
                for ent in fq:
                    if dl_le(ent[0], bound):
                        ent[1]()
                    else:
                        rest.append(ent)
                fq[:] = rest

            proj_qk(0, 0)
            proj_qk(0, 1)
            proj_v(0)
            proj_v(1)

            all_pairs = [(i, m) for i in range(NCH) for m in range(2 * i + 2)]
            prev = None
            for (i, m) in all_pairs:
                if m == 0:
                    flush(('pst', i))
                wst = pst_exp(i, m)
                if prev is not None:
                    flush(('po', prev[0], prev[1]))
                    po_pair(*prev)
                if fq:
                    fq.pop(0)[1]()
                prev = (i, m, wst)
            flush(('po', NCH - 1, 10 ** 6))
            po_pair(*prev)
            tail(NCH - 1)

    nc.compile()
    return nc


def _host_prep(x, Wq, Wk, Wv):
    bf = ml_dtypes.bfloat16
    xb = np.ascontiguousarray(x).astype(bf)
    w = np.concatenate([Wq, Wk, Wv], axis=1)          # [1024, 192]
    wqkv = np.ascontiguousarray(
        w.reshape(NCT, 128, 192).transpose(1, 0, 2).reshape(128, NCT * 192).T
    ).astype(bf)
    return xb, wqkv


def kernel(x, Wq, Wk, Wv, trace=False):
    x = np.asarray(x, dtype=np.float32)
    Wq = np.asarray(Wq, dtype=np.float32)
    Wk = np.asarray(Wk, dtype=np.float32)
    Wv = np.asarray(Wv, dtype=np.float32)

    if "nc" not in _CACHE:
        _CACHE["nc"] = build()
    nc = _CACHE["nc"]

    xb, wqkv = _host_prep(x, Wq, Wk, Wv)
    in_maps = [{"xb": xb[b], "wqkv": wqkv} for b in range(B)]
    try:
        res = run_bass_kernel_spmd(nc, in_maps, core_ids=list(range(B)), trace=trace)
    except ModuleNotFoundError:
        res = run_bass_kernel_spmd(nc, in_maps, core_ids=list(range(B)))
    out = np.stack([np.asarray(r["out"]).astype(np.float32) for r in res.results], axis=0)
    kernel.last_exec_time_ns = res.exec_time_ns
    kernel.last_results = res
    return out
